# revision 8
# baseline (speedup 1.0000x reference)
"""Multi-head attention (B=2, S=2048, D=1024, H=16) on 8 Trainium2 NeuronCores.

Sharding: core c handles batch b = c//4 and head group g = c%4 (4 heads, 256
model dims).  Each core computes q/k/v projections for its heads, attention,
and a partial output projection (row-parallel over its 256 head dims); the
host sums the 4 partials per batch and adds the bias.

All activations live in transposed layouts so the tensor engine contraction
dim always sits on SBUF partitions:
  xT  [d, s]  (host pre-transposed)
  qT/kT [e, s] per head-pair tile (heads at partitions 0-63 / 64-127)
  v   [s, e+1] per (s-block, head) with a trailing ones column so the PV
      matmul's 65th output row is the softmax denominator.
Scores are computed transposed [ks, qs]; softmax needs no max-subtraction
(scores ~ N(0,1)) and no free-dim reduction: exp runs elementwise on ACT and
the denominator rides the PV matmul.  Normalization broadcasts 1/denominator
via a DRAM round-trip (partition-broadcast DMA) and one DVE multiply.

Matmuls use float32r (~1.5e-4 rel err, full PE rate for free dims >= 256).
"""

import os
import sys

import numpy as np

for _p in ("/opt/trn_rl_repo", "/root/.axon_site/_ro/trn_rl_repo"):
    if os.path.isdir(_p) and _p not in sys.path:
        sys.path.insert(0, _p)

import bass_rust
import concourse.bass as bass
import concourse.mybir as mybir
import concourse.tile as tile
from concourse.bass_utils import run_bass_kernel_spmd
from concourse.vector_clock import ScopedClock, VectorClock
from contextlib import ExitStack

F32 = mybir.dt.float32
F32R = mybir.dt.float32r
EXP = mybir.ActivationFunctionType.Exp

B = 2
S = 2048
D = 1024
H = 16
HD = 64
NCORES = 8
GROUPS = 4          # head groups (cores per batch)
HG = H // GROUPS    # heads per core = 4
E = HG * HD         # head dims per core = 256
KT = D // 128       # contraction tiles over model dim = 8
SB = S // 128       # s blocks = 16
QB = S // 512       # 512-wide qs blocks = 4

_carrier_counter = [0]


def _split_multi_waits(ordered):
    """This walrus build allows one sync wait per instruction; Tile's wait
    assignment can attach several.  Hoist extras onto same-engine InstNoOp
    carriers placed immediately before the instruction."""
    for bb_name, insts in ordered.items():
        new_list = []
        for inst in insts:
            si = inst.sync_info
            waits = list(si.on_wait) if si is not None else []
            if len(waits) > 1:
                for w in waits[:-1]:
                    _carrier_counter[0] += 1
                    carrier = mybir.InstNoOp(
                        name=f"I-waitc-{_carrier_counter[0]}", ins=[], outs=[]
                    )
                    carrier.engine = inst.engine
                    carrier.sync_info = bass_rust.SyncInfo(on_wait=[w], on_update=[])
                    new_list.append(carrier)
                inst.sync_info = bass_rust.SyncInfo(
                    on_wait=[waits[-1]],
                    on_update=list(si.on_update) if si is not None else [],
                )
            new_list.append(inst)
        ordered[bb_name] = new_list


class _TileContext(tile.TileContext):
    """TileContext adapted to the one-sync-wait-per-instruction walrus."""

    def _lower_ordered_insts(self, ordered):
        _split_multi_waits(ordered)
        return super()._lower_ordered_insts(ordered)

    def _drain_and_barrier(self, tick_clock, wait_clock):
        gc = tick_clock.global_clock
        for proc in range(len(gc)):
            if gc[proc] <= 0:
                continue
            cur = VectorClock([0 if i == proc else gc[i] for i in range(len(gc))])
            nop = self.nc.sync.nop()
            wait_clock.add_sem_waits(
                nop.ins, ScopedClock({None: gc}), ScopedClock({None: cur})
            )
        drain_inst = self.nc.sync.drain()
        wait_clock.add_sem_waits(
            drain_inst.ins, ScopedClock({None: gc}), ScopedClock({None: gc.copy()})
        )
        self.nc.all_engine_barrier()
        assert self.sems is not None
        popped = self.nc._tile_sem_poison_stack.pop()
        assert popped is self._sem_poison
        self.nc.clear_and_free_semaphores(list(self.sems.allocated().values()))
        self.nc.all_engine_barrier()


def build_nc():
    nc = bass.Bass()
    xT = nc.declare_dram_parameter("xT", [D, S], F32R, isOutput=False)
    wqT = nc.declare_dram_parameter("wqT", [D, E], F32R, isOutput=False)
    wkT = nc.declare_dram_parameter("wkT", [D, E], F32R, isOutput=False)
    wvT = nc.declare_dram_parameter("wvT", [D, E], F32R, isOutput=False)
    woT = nc.declare_dram_parameter("woT", [E, D], F32R, isOutput=False)
    out = nc.declare_dram_parameter("out_partial", [S, D], F32, isOutput=True)
    ones_d = nc.declare_dram_parameter("ones_d", [128, SB * HG], F32R, isOutput=False)
    den_d = nc.dram_tensor("den_scratch", [HG, S], F32)

    with _TileContext(nc) as tc, ExitStack() as ctx:
        # ---- persistent activation tiles (live across phases) ----
        act_pool = ctx.enter_context(tc.tile_pool(name="acts", bufs=1))
        qT_sb = [act_pool.tile([128, S], F32R, tag=f"qT{m}", name=f"qT{m}") for m in range(2)]
        kT_sb = [act_pool.tile([128, S], F32R, tag=f"kT{m}", name=f"kT{m}") for m in range(2)]
        v_sb = act_pool.tile([128, SB, HG, HD + 1], F32R, tag="v")
        wo_sb = [act_pool.tile([64, D], F32R, tag=f"wo{h}", name=f"wo{h}") for h in range(HG)]

        # ---- phase 1: load inputs, project q/k/v ----
        with ExitStack() as c1:
            in_pool = c1.enter_context(tc.tile_pool(name="ins", bufs=1))
            ps_qk = c1.enter_context(tc.tile_pool(name="ps_qk", bufs=2, space="PSUM"))
            ps_v = c1.enter_context(tc.tile_pool(name="ps_v", bufs=2, space="PSUM"))

            x_sb = in_pool.tile([128, KT, S], F32R, tag="x")
            wq_sb = in_pool.tile([128, KT, E], F32R, tag="wq")
            wk_sb = in_pool.tile([128, KT, E], F32R, tag="wk")
            wv_sb = in_pool.tile([128, KT, E], F32R, tag="wv")
            for k in range(KT):
                nc.sync.dma_start(x_sb[:, k, :], xT[k * 128:(k + 1) * 128, :])
                nc.sync.dma_start(wq_sb[:, k, :], wqT[k * 128:(k + 1) * 128, :])
                nc.sync.dma_start(wk_sb[:, k, :], wkT[k * 128:(k + 1) * 128, :])
                nc.sync.dma_start(wv_sb[:, k, :], wvT[k * 128:(k + 1) * 128, :])
            for h in range(HG):
                nc.sync.dma_start(wo_sb[h][:, :], woT[h * 64:(h + 1) * 64, :])

            # ones column for the softmax-denominator rows of v
            nc.sync.dma_start(
                v_sb[:, :, :, HD],
                ones_d[:, :].rearrange("p (s h) -> p s h", s=SB),
            )

            # qT / kT: W-stationary, out [e(128), s]
            for w_sb, dst in ((wq_sb, qT_sb), (wk_sb, kT_sb)):
                for m in range(2):
                    for nb in range(QB):
                        ps = ps_qk.tile([128, 512], F32)
                        for k in range(KT):
                            nc.tensor.matmul(
                                ps[:],
                                w_sb[:, k, m * 128:(m + 1) * 128],
                                x_sb[:, k, nb * 512:(nb + 1) * 512],
                                start=(k == 0),
                                stop=(k == KT - 1),
                            )
                        nc.vector.tensor_copy(
                            dst[m][:, nb * 512:(nb + 1) * 512], ps[:]
                        )
            # v: x-stationary, out [s(128), e]
            for sb in range(SB):
                ps = ps_v.tile([128, E], F32)
                for k in range(KT):
                    nc.tensor.matmul(
                        ps[:],
                        x_sb[:, k, sb * 128:(sb + 1) * 128],
                        wv_sb[:, k, :],
                        start=(k == 0),
                        stop=(k == KT - 1),
                    )
                nc.vector.tensor_copy(
                    v_sb[:, sb, :, 0:HD],
                    ps[:].rearrange("p (h e) -> p h e", h=HG),
                )

        # ---- phase 2: attention per head-pair (m), per qs-half ----
        attn_pool = ctx.enter_context(tc.tile_pool(name="attn", bufs=1))
        attn_sb = [attn_pool.tile([64, S], F32R, tag=f"at{h}", name=f"at{h}") for h in range(HG)]
        with ExitStack() as c2:
            p_pool = c2.enter_context(tc.tile_pool(name="pexp", bufs=3))
            rb_pool = c2.enter_context(tc.tile_pool(name="rbc", bufs=2))
            den_pool = c2.enter_context(tc.tile_pool(name="den", bufs=2))
            ps_pv = c2.enter_context(tc.tile_pool(name="ps_pv", bufs=1, space="PSUM"))
            ps_sc = c2.enter_context(tc.tile_pool(name="ps_sc", bufs=1, space="PSUM"))

            for m in range(2):
                for qh in range(2):  # qs halves of 1024
                    out_ps = [
                        [ps_pv.tile([128, 512], F32, tag=f"pv{r}{qq}", name=f"pv{r}{qq}") for qq in range(2)]
                        for r in range(2)
                    ]
                    for ksb in range(SB):
                        sc = [ps_sc.tile([128, 1024], F32, tag=f"sc{r}", name=f"sc{r}") for r in range(2)]
                        for qq in range(2):
                            for r in range(2):
                                nc.tensor.matmul(
                                    sc[r][:, qq * 512:(qq + 1) * 512],
                                    kT_sb[m][64 * r:64 * r + 64,
                                             ksb * 128:(ksb + 1) * 128],
                                    qT_sb[m][64 * r:64 * r + 64,
                                             qh * 1024 + qq * 512:
                                             qh * 1024 + (qq + 1) * 512],
                                    start=True,
                                    stop=True,
                                )
                        pt = [None, None]
                        for r in range(2):
                            pt[r] = p_pool.tile([128, 1024], F32R, tag="p", name=f"p{r}")
                            nc.scalar.activation(pt[r][:], sc[r][:], EXP)
                        for r in range(2):
                            for qq in range(2):
                                nc.tensor.matmul(
                                    out_ps[r][qq][0:HD + 1, :],
                                    v_sb[:, ksb, 2 * m + r, :],
                                    pt[r][:, qq * 512:(qq + 1) * 512],
                                    start=(ksb == 0),
                                    stop=(ksb == SB - 1),
                                )
                    # normalize: attn = attnU * (1/denom), denom = row 64
                    for r in range(2):
                        h = 2 * m + r
                        den = den_pool.tile([128, 1024], F32, tag="den")
                        for qq in range(2):
                            nc.vector.tensor_copy(
                                den[64:65, qq * 512:(qq + 1) * 512],
                                out_ps[r][qq][64:65, :],
                            )
                        nc.vector.reciprocal(den[64:65, :], den[64:65, :])
                        nc.sync.dma_start(
                            den_d[h:h + 1, qh * 1024:(qh + 1) * 1024], den[64:65, :]
                        )
                        rb = rb_pool.tile([64, 1024], F32, tag="rb")
                        nc.sync.dma_start(
                            rb[:, :],
                            den_d[h:h + 1, qh * 1024:(qh + 1) * 1024]
                            .to_broadcast((64, 1024)),
                        )
                        for qq in range(2):
                            qs0 = qh * 1024 + qq * 512
                            nc.vector.tensor_mul(
                                attn_sb[h][:, qs0:qs0 + 512],
                                out_ps[r][qq][0:64, :],
                                rb[:, qq * 512:(qq + 1) * 512],
                            )

        # ---- phase 3: output projection (row-parallel partial) ----
        with ExitStack() as c3:
            stage_pool = c3.enter_context(tc.tile_pool(name="ostage", bufs=3))
            ps_o = c3.enter_context(
                tc.tile_pool(name="ps_o", bufs=2, space="PSUM")
            )
            for sb in range(SB):
                for nb in range(2):
                    ps = ps_o.tile([128, 512], F32)
                    for h in range(HG):
                        nc.tensor.matmul(
                            ps[:],
                            attn_sb[h][:, sb * 128:(sb + 1) * 128],
                            wo_sb[h][:, nb * 512:(nb + 1) * 512],
                            start=(h == 0),
                            stop=(h == HG - 1),
                        )
                    st = stage_pool.tile([128, 512], F32, tag="st")
                    nc.vector.tensor_copy(st[:], ps[:])
                    nc.sync.dma_start(
                        out[sb * 128:(sb + 1) * 128, nb * 512:(nb + 1) * 512],
                        st[:],
                    )
    return nc


_NC_CACHE = None


def _get_nc():
    global _NC_CACHE
    if _NC_CACHE is None:
        _NC_CACHE = build_nc()
    return _NC_CACHE


def _shard_inputs(x, Wq, Wk, Wv, Wo):
    scale = np.float32(1.0 / np.sqrt(HD))
    global _ONES
    _ONES = np.ones((128, SB * HG), dtype=np.float32)
    in_maps = []
    xT_b = [np.ascontiguousarray(x[b].T) for b in range(B)]
    for c in range(NCORES):
        b, g = divmod(c, GROUPS)
        sl = slice(g * E, (g + 1) * E)
        in_maps.append({
            "ones_d": _ONES,
            "xT": xT_b[b],
            "wqT": np.ascontiguousarray(Wq[sl, :].T * scale),
            "wkT": np.ascontiguousarray(Wk[sl, :].T),
            "wvT": np.ascontiguousarray(Wv[sl, :].T),
            "woT": np.ascontiguousarray(Wo[:, sl].T),
        })
    return in_maps


def kernel(x, Wq, Wk, Wv, Wo, bo):
    x = np.asarray(x, dtype=np.float32)
    nc = _get_nc()
    in_maps = _shard_inputs(
        x,
        np.asarray(Wq, dtype=np.float32),
        np.asarray(Wk, dtype=np.float32),
        np.asarray(Wv, dtype=np.float32),
        np.asarray(Wo, dtype=np.float32),
    )
    res = run_bass_kernel_spmd(nc, in_maps, list(range(NCORES)))
    bo = np.asarray(bo, dtype=np.float32)
    out = np.empty((B, S, D), dtype=np.float32)
    for b in range(B):
        acc = np.zeros((S, D), dtype=np.float64)
        for g in range(GROUPS):
            acc += res.results[b * GROUPS + g]["out_partial"]
        out[b] = (acc + bo.astype(np.float64)).astype(np.float32)
    return out
